# revision 1
# baseline (speedup 1.0000x reference)
"""Trainium2 Bass kernel for nn_Attention_49813030699234.

Conv-attention block: depthwise 3x3 convs -> q/k/v linear projections ->
8-head attention -> output projection.  B=4, N=2304 (48x48), C=256, 8 heads.

Sharding: 8 cores = 4 batches x 2 head-groups (4 heads each).  The depthwise
conv is folded into the projection weights on the host (9 shifted matmuls
accumulating in PSUM against a zero-padded channel-major image).

Device dataflow (all matmul inputs bf16, PSUM accumulation fp32):
  fused conv+proj -> qT/kT/vT [128, N] (d-major) -> v transposed to
  token-major tiles -> transposed-score attention: scoresT = kT.T-tiles x qT
  (16-way PE tile packing), exp on ACT in fp32, then p = exp(s)-1 cast to
  bf16 on DVE (exp(s) is ~1.0 +- 1e-4 here, so subtracting 1 before the
  bf16 cast preserves the attention signal exactly; the "+1" parts are
  restored exactly via out += V1 = sum_t v[t] and S = 2304 + sum_t p).
  attn@v and softmax denominators via ones-matmul accumulate in PSUM across
  token chunks; normalize + partial output projection per query slice.
Host sums the two head-group partials per batch and adds bias.
"""

import numpy as np

B, N, C, NH = 4, 2304, 256, 8
H = 48          # spatial side (N = H*H)
PAD = H + 2     # zero-padded side
HD = C // NH    # 32 head dim
G = 2           # head groups (cores per batch)
SCALE = C ** -0.5
NT = N // 128   # 18 key/token chunks
# query slices (<=512 free dim per matmul: one PSUM bank)
QS = [(0, 512), (512, 512), (1024, 512), (1536, 512), (2048, 256)]
# token row-blocks for the projection (rows of the 48x48 grid; 48*R <= 480)
TB = [(0, 10), (10, 10), (20, 10), (30, 10), (40, 8)]

_NC = None  # cached compiled Bass program (same program for all cores)


def _build_bass():
    import concourse.bacc as bacc
    import concourse.mybir as mybir
    import concourse.tile as tile
    from concourse.masks import make_identity

    f32 = mybir.dt.float32
    bf16 = mybir.dt.bfloat16
    Exp = mybir.ActivationFunctionType.Exp

    nc = bacc.Bacc("TRN2")
    xp = nc.dram_tensor("xp", [128, 2, PAD, PAD], bf16, kind="ExternalInput")
    wt = nc.dram_tensor("wt", [128, 54, 128], bf16, kind="ExternalInput")
    wpt = nc.dram_tensor("wpt", [128, C], bf16, kind="ExternalInput")
    yt = nc.dram_tensor("yt", [C, N], f32, kind="ExternalOutput")

    with tile.TileContext(nc) as tc:
        with tc.tile_pool(name="const", bufs=1) as cp:
            xp_sb = [cp.tile([128, PAD, PAD], bf16, tag=f"xp{cc}", name=f"xp_sb{cc}") for cc in range(2)]
            wt_sb = cp.tile([128, 54, 128], bf16, tag="wt")
            wpt_hp = [cp.tile([64, C], bf16, tag=f"wpt{hp}", name=f"wpt_hp{hp}")
                      for hp in range(2)]
            ident = cp.tile([128, 128], bf16, tag="ident")
            ones = cp.tile([128, 32], bf16, tag="ones")
            qT = cp.tile([128, N], bf16, tag="qT")
            kT = cp.tile([128, N], bf16, tag="kT")
            vT = cp.tile([128, N], bf16, tag="vT")
            vtok = cp.tile([128, N], bf16, tag="vtok")
            v1_sb = cp.tile([128, 1], f32, tag="v1_sb")

            for cc in range(2):
                nc.sync.dma_start(out=xp_sb[cc], in_=xp[:, cc])
            nc.sync.dma_start(out=wt_sb, in_=wt[:])
            for hp in range(2):
                nc.sync.dma_start(out=wpt_hp[hp], in_=wpt[64 * hp: 64 * hp + 64])
            make_identity(nc, ident)
            nc.vector.memset(ones, 1.0)

            # ---- fused depthwise-conv + projection: qT/kT/vT [128, N] ----
            # dst[j, tok] = sum_{cc,tap} wt[(p,tap,cc)][c, j]^T x_pad[c, tok+tap]
            with tc.tile_pool(name="psA", bufs=2, space="PSUM") as psA:
                # keep the PE busy (and HAM un-throttled) while inputs DMA in
                psw = psA.tile([128, 480], f32, tag="proj", name="psw")
                for w in range(40):
                    nc.tensor.matmul(psw[:, 0:128], ident, ident,
                                     start=(w == 0), stop=(w == 39))
                for p, dst in enumerate([qT, kT, vT]):
                    for (r0, R) in TB:
                        nw = 48 * R
                        ps = psA.tile([128, 480], f32, tag="proj")
                        k = 0
                        for cc in range(2):
                            for tap in range(9):
                                dy, dx = divmod(tap, 3)
                                idx = (p * 9 + tap) * 2 + cc
                                nc.tensor.matmul(
                                    ps[:, :nw],
                                    wt_sb[:, idx],
                                    xp_sb[cc][:, r0 + dy: r0 + dy + R, dx: dx + 48],
                                    start=(k == 0), stop=(k == 17),
                                )
                                k += 1
                        nc.vector.tensor_copy(
                            out=dst[:, 48 * r0: 48 * r0 + nw], in_=ps[:, :nw])

                # ---- v -> token-major tiles: vtok[:, 128t+32h+d] ----
                for t in range(NT):
                    ps = psA.tile([128, 128], bf16, tag="vt")
                    nc.tensor.transpose(ps, vT[:, 128 * t: 128 * (t + 1)], ident)
                    nc.vector.tensor_copy(
                        out=vtok[:, 128 * t: 128 * (t + 1)], in_=ps)

                # ---- V1[d] = sum_t v[t, d] (restores the "+1" of exp) ----
                ps_v1 = psA.tile([128, 1], f32, tag="v1")
                for t in range(NT):
                    nc.tensor.matmul(
                        ps_v1, vtok[:, 128 * t: 128 * (t + 1)], ones[:, 0:1],
                        start=(t == 0), stop=(t == NT - 1))
                nc.vector.tensor_copy(out=v1_sb, in_=ps_v1)

            # ---- attention (transposed scores) + output projection ----
            # Head pairs hp in {0,1}: heads {2hp, 2hp+1}.  Per (q-slice, hp):
            # acc tile rows = [out_h0 | out_h1 | S_h0 | S_h1] (32 rows each),
            # written by 4 concurrent col-tiled matmuls per token chunk.
            with (
                tc.tile_pool(name="sc", bufs=2, space="PSUM") as scp,
                tc.tile_pool(name="acc", bufs=3, space="PSUM") as accp,
                tc.tile_pool(name="py", bufs=1, space="PSUM") as pyp,
                tc.tile_pool(name="ex32", bufs=4) as ex32p,
                tc.tile_pool(name="pb", bufs=6) as pbp,
                tc.tile_pool(name="ob", bufs=4) as obp,
                tc.tile_pool(name="yb", bufs=4) as ybp,
            ):
                def emit_qk(q0, qn, hp, t, sc):
                    for h in range(2):
                        ha = 2 * hp + h
                        for j in range(4):
                            nc.tensor.matmul(
                                sc[32 * j: 32 * j + 32, h, :qn],
                                kT[32 * ha: 32 * ha + 32,
                                   128 * t + 32 * j: 128 * t + 32 * j + 32],
                                qT[32 * ha: 32 * ha + 32, q0: q0 + qn],
                                start=True, stop=True,
                                tile_position=(32 * ha, 32 * j),
                            )

                def emit_exp_sub(qn, sc):
                    ex = ex32p.tile([128, 2, 512], f32, tag="ex", name="ex")
                    nc.scalar.activation(
                        out=ex[:, :, :qn], in_=sc[:, :, :qn],
                        func=Exp, scale=SCALE)
                    pb = pbp.tile([128, 2, 512], bf16, tag="pb", name="pb")
                    if qn == 512:
                        nc.vector.tensor_scalar_add(
                            out=pb.rearrange("p a b -> p (a b)"),
                            in0=ex.rearrange("p a b -> p (a b)"),
                            scalar1=-1.0)
                    else:
                        for h in range(2):
                            nc.vector.tensor_scalar_add(
                                out=pb[:, h, :qn], in0=ex[:, h, :qn],
                                scalar1=-1.0)
                    return pb

                def emit_av(qn, hp, t, pb, acc):
                    first, last = (t == 0), (t == NT - 1)
                    for h in range(2):
                        ha = 2 * hp + h
                        nc.tensor.matmul(
                            acc[32 * h: 32 * h + 32, :qn],
                            vtok[:, 128 * t + 32 * ha: 128 * t + 32 * ha + 32],
                            pb[:, h, :qn],
                            start=first, stop=last,
                            tile_position=(0, 32 * h),
                        )
                        nc.tensor.matmul(
                            acc[64 + 32 * h: 96 + 32 * h, :qn],
                            ones,
                            pb[:, h, :qn],
                            start=first, stop=last,
                            tile_position=(0, 64 + 32 * h),
                        )

                pending_proj = [None]

                def emit_pending():
                    if pending_proj[0] is not None:
                        pending_proj[0]()
                        pending_proj[0] = None

                groups = [(q0, qn, hp) for (q0, qn) in QS for hp in range(2)]
                total = len(groups) * NT
                accs, sc_q, pb_q = {}, {}, {}
                obs_by_qs = {}

                def emit_normalize(q0, qn, hp, gi):
                    acc = accs.pop(gi)
                    ob32 = obp.tile([128, 512], f32, tag="ob32", name="ob32")
                    nc.vector.tensor_scalar_add(
                        out=ob32[0:64, :qn], in0=acc[0:64, :qn],
                        scalar1=v1_sb[64 * hp: 64 * hp + 64])
                    nc.vector.tensor_scalar_add(
                        out=ob32[64:128, :qn], in0=acc[64:128, :qn],
                        scalar1=float(N))
                    rc = obp.tile([64, 512], f32, tag="rc", name="rc")
                    nc.vector.reciprocal(
                        out=rc[:, :qn], in_=ob32[64:128, :qn])
                    ob = obp.tile([64, 512], bf16, tag="ob", name="ob")
                    nc.vector.tensor_mul(
                        ob[:, :qn], ob32[0:64, :qn], rc[:, :qn])
                    obs_by_qs.setdefault(q0, []).append(ob)
                    if hp == 1:
                        def _proj(q0=q0, qn=qn):
                            obs = obs_by_qs[q0]
                            for j in range(2):
                                py = pyp.tile([128, 512], f32, tag="py", name="py")
                                for hp2 in range(2):
                                    nc.tensor.matmul(
                                        py[:, :qn],
                                        wpt_hp[hp2][:, 128 * j: 128 * j + 128],
                                        obs[hp2][:, :qn],
                                        start=(hp2 == 0), stop=(hp2 == 1))
                                yb = ybp.tile([128, 512], f32, tag="yb", name="yb")
                                nc.scalar.copy(out=yb[:, :qn], in_=py[:, :qn])
                                nc.sync.dma_start(
                                    out=yt[128 * j: 128 * j + 128, q0: q0 + qn],
                                    in_=yb[:, :qn])
                        pending_proj[0] = _proj

                # one continuous 2-stage software pipeline over every
                # (q-slice, head-pair, token-chunk): QK(c) | exp/sub(c-1) | AV(c-2)
                for c in range(total + 3):
                    if c < total:
                        (q0, qn, hp), gi, t = groups[c // NT], c // NT, c % NT
                        if t == 0:
                            accs[gi] = accp.tile([128, 512], f32, tag="acc", name="acc")
                        if t == 4:
                            emit_pending()
                        sc = scp.tile([128, 2, 512], f32, tag="sc", name="sc")
                        emit_qk(q0, qn, hp, t, sc)
                        sc_q[c] = sc
                    if 1 <= c <= total:
                        (q0, qn, hp), gi, t = groups[(c - 1) // NT], (c - 1) // NT, (c - 1) % NT
                        pb_q[c - 1] = emit_exp_sub(qn, sc_q.pop(c - 1))
                    if c >= 3:
                        (q0, qn, hp), gi, t = groups[(c - 3) // NT], (c - 3) // NT, (c - 3) % NT
                        emit_av(qn, hp, t, pb_q.pop(c - 3), accs[gi])
                        if t == NT - 1:
                            emit_normalize(q0, qn, hp, gi)
                emit_pending()
    nc.compile()
    return nc


def _get_nc():
    global _NC
    if _NC is None:
        _NC = _build_bass()
    return _NC


LAST = {"exec_time_ns": None, "results": None}


def kernel(**inputs):
    import ml_dtypes
    bf16 = ml_dtypes.bfloat16

    x = np.asarray(inputs["x"], np.float32)
    convs = {p: np.asarray(inputs[f"w{p}_conv"], np.float32) for p in "qkv"}
    Ws = {p: np.asarray(inputs[f"W{p}"], np.float32) for p in "qkv"}
    Wp = np.asarray(inputs["Wp"], np.float32)
    bp = np.asarray(inputs["bp"], np.float32)

    # x [B, N, C] -> zero-padded channel-major [B, 128, 2, PAD, PAD]
    xt = x.transpose(0, 2, 1).reshape(B, C, H, H)
    xpad = np.zeros((B, C, PAD, PAD), np.float32)
    xpad[:, :, 1:-1, 1:-1] = xt
    xp_all = xpad.reshape(B, 2, 128, PAD, PAD).transpose(0, 2, 1, 3, 4)

    in_maps = []
    for core in range(8):
        b, g = divmod(core, 2)
        # fold depthwise conv taps into projection weights (lhsT layout [c, j])
        wt_host = np.empty((128, 54, 128), np.float32)
        for pi, p in enumerate("qkv"):
            Wg = Ws[p][128 * g: 128 * (g + 1), :]      # [128 j, 256 c]
            cv = convs[p][:, 0]                        # [256 c, 3, 3]
            for tap in range(9):
                dy, dx = divmod(tap, 3)
                wtile = (Wg * cv[:, dy, dx][None, :]).T  # [256 c, 128 j]
                for cc in range(2):
                    idx = (pi * 9 + tap) * 2 + cc
                    wt_host[:, idx, :] = wtile[128 * cc: 128 * (cc + 1), :]
        wpt = np.ascontiguousarray(Wp[:, 128 * g: 128 * (g + 1)].T)
        in_maps.append({
            "xp": np.ascontiguousarray(xp_all[b]).astype(bf16),
            "wt": wt_host.astype(bf16),
            "wpt": wpt.astype(bf16),
        })

    from concourse.bass_utils import run_bass_kernel_spmd
    import os
    trace = bool(os.environ.get("KERNEL_TRACE"))
    out = run_bass_kernel_spmd(_get_nc(), in_maps, list(range(8)), trace=trace)
    LAST["exec_time_ns"] = out.exec_time_ns
    LAST["mean_exec_time_ns"] = getattr(out, "mean_exec_time_ns", None)
    res = out.results

    y = np.empty((B, N, C), np.float32)
    for b in range(B):
        ytp = res[2 * b]["yt"] + res[2 * b + 1]["yt"]   # [C, N]
        y[b] = ytp.T + bp[None, :]
    return y

